# revision 19
# baseline (speedup 1.0000x reference)
"""Multi-head attention on 8 TRN2 NeuronCores.

Problem: x[2, 2048, 1024], w_qkv[1024, 3072], w_out[1024, 1024] (f32).
  qkv = x @ w_qkv; q,k,v per 16 heads of dim 64; softmax(q k^T / 8) v; out proj.

Sharding: 16 heads split 8 ways (one head-PAIR per core, both batches on
every core).  Each core computes q^T/k^T/v for its 2 heads over all
B*L = 4096 rows, runs attention, then an 8-rank AllToAll exchanges
(head-pair -> (batch, L/4-chunk)) so each core finishes the output
projection for its own 512 output rows with all 16 heads present.  The
AllToAll is split into two half-exchanges (one per local head): the first
fires halfway through attention and fully overlaps the remaining compute.

Layout: scores are computed TRANSPOSED (S^T[m, l] tiles) so softmax's sum
runs over the partition axis -- free via a ones-column appended to v in
the attn@v matmul (out rows = [o^T; colsums]).  Score matmuls come in
row-group-packed pairs: the even m-tile runs in PE rows 0:64 while the
odd m-tile runs concurrently in rows 64:128, via two q/k SBUF images --
"direct" (head0 top / head1 bottom, one fat DVE/ScalarE copy from the
projection PSUM) and "swapped" (halves exchanged by SBUF->SBUF DMA).
exp() splits across ScalarE (spline exp, 5 of 8 m-tile pairs) and the DVE
(3 of 8 via the one-op Schraudolph bit-trick); q is pre-scaled so both
read the same PSUM.  Normalization is transpose-free: the colsum row is
copied to partition 0 (the custom-DVE fast reciprocal cannot shift
partitions), reciprocal'd, broadcast over 64 partitions on gpsimd, and
multiplied into the output in one DVE op.  The attention loop is
software-pipelined: each chunk's PV matmul chain is emitted a chunk
behind its scores/exp so it never waits on the exp engines.  Chunks that
run while a collective occupies the gpsimd queue park their numerator/
denominator in SBUF and defer the norm (the broadcast would stall in the
gpsimd FIFO behind the collective and drag the DVE with it).

Tail: the out-projection is split by head parity.  Even heads arrive with
the first AllToAll, so their half of the contraction (partials parked in
SBUF, aliasing the parked-norm slots) runs during the second AllToAll's
transfer window; a chain of heater matmuls keeps the PE's HAM clock gate
warm across the collective so the odd-head half + final add run at full
clock instead of 1.2 GHz.

Compute dtype bf16 (f32 accumulation in PSUM); output returned bf16 and
upcast on host.
"""

import sys
import types

sys.path.insert(0, "/opt/trn_rl_repo")

import numpy as np
import ml_dtypes

import concourse.bass as bass
import concourse.mybir as mybir
import concourse.tile as tile
from concourse import bacc
from concourse import bass_utils

# If the image's antenv lacks the axon_hooks module, run_bass_kernel_spmd's
# trace path (reachable via BASS_TRACE=1) would die on import.  Provide the
# registry so tracing degrades gracefully instead (hook stays None unless
# trn_boot registered one).
try:
    import antenv.axon_hooks  # noqa: F401
except ImportError:
    _hooks = types.ModuleType("antenv.axon_hooks")
    _hooks._hook = None
    _hooks.set_axon_ntff_profile_hook = (
        lambda h: setattr(_hooks, "_hook", h))
    _hooks.get_axon_ntff_profile_hook = lambda: _hooks._hook
    sys.modules["antenv.axon_hooks"] = _hooks

# Artifact upload needs bucket credentials; fall back to the local dir so a
# traced run in a sandboxed container still completes.
_orig_upload = bass_utils.upload_artifacts


def _safe_upload(tmpdir):
    try:
        return _orig_upload(tmpdir)
    except Exception:
        return tmpdir


bass_utils.upload_artifacts = _safe_upload

B, L, D, H, DH = 2, 2048, 1024, 16, 64
BL = B * L  # 4096
SCALE = DH ** -0.5
N_CORES = 8
BF16 = mybir.dt.bfloat16
F32 = mybir.dt.float32
Exp = mybir.ActivationFunctionType.Exp

# exp on two engines: ScalarE evaluates the spline exp; the DVE handles the
# last DVE_MPS of every 16 key-tiles with a one-op Schraudolph bit-trick --
# bf16(int16(st + B)) ~= exp(st/A).  q is pre-scaled by A so the scores
# PSUM is already in bits-space; a uniform bits offset is a global scale
# on the softmax weights and cancels in normalization, so only the ~1.8%
# rms sawtooth remains, on the DVE-covered fraction of the keys.
SCH_A = 128 * 1.4426950408889634 * SCALE      # 23.0831
SCH_B = 16248.636                             # host-calibrated, zero mean bias
SCH_SCALE = SCALE / SCH_A                     # ScalarE: exp(st * this)
DVE_MPS = (5, 6, 7)                           # mp pairs handled by the DVE

KT = D // 128          # 8 k-tiles over the model dim
MT = L // 128          # 16 m-tiles per batch
LC = L // 512          # 4 l-chunks of 512 per batch
VT = BL // 128         # 32 v row-tiles over (b, l)
N_HEAT = 58            # heater matmuls bridging the second AllToAll


def _build():
    nc = bacc.Bacc("TRN2", target_bir_lowering=False, debug=False,
                   num_devices=N_CORES)
    xT_ext = nc.declare_dram_parameter("xT", [D, BL], BF16, isOutput=False)
    wqk_ext = nc.declare_dram_parameter("wqk", [D, 256], BF16, isOutput=False)
    wv_ext = nc.declare_dram_parameter("wv", [D, 128], BF16, isOutput=False)
    # w_out rows pre-permuted on the host: first 512 = even heads
    # (0,2,..,14), last 512 = odd heads -- so the contraction splits into
    # a half that only needs the first AllToAll and a half that needs the
    # second.
    wout_ext = nc.declare_dram_parameter("wout", [D, D], BF16, isOutput=False)
    out_ext = nc.declare_dram_parameter("out", [512, D], BF16, isOutput=True)

    with tile.TileContext(nc) as tc:
        with (
            tc.tile_pool(name="big", bufs=1) as big,
            tc.tile_pool(name="pt", bufs=3) as ptp,
            tc.tile_pool(name="small", bufs=2) as small,
            tc.tile_pool(name="ser", bufs=1) as ser,
            tc.tile_pool(name="psum_st", bufs=3, space="PSUM") as pst,
            tc.tile_pool(name="psum_ov", bufs=2, space="PSUM") as pov,
            tc.tile_pool(name="dram", bufs=1, space="DRAM") as dram,
        ):
            # ---- static SBUF tensors ----
            xT_t = [big.tile([128, BL], BF16, tag=f"xT{k}", name=f"xT{k}")
                    for k in range(KT)]
            wqk_t = [big.tile([128, 256], BF16, tag=f"wqk{k}",
                              name=f"wqk{k}") for k in range(KT)]
            wv_t = [big.tile([128, 128], BF16, tag=f"wv{k}", name=f"wv{k}")
                    for k in range(KT)]
            for k in range(KT):
                nc.sync.dma_start(
                    xT_t[k][:, 0:512], xT_ext[k * 128:(k + 1) * 128, 0:512])
                nc.sync.dma_start(wqk_t[k][:], wqk_ext[k * 128:(k + 1) * 128, :])
            # fat chunks: >=1.5KB per partition line for DMA efficiency
            for k in range(KT):
                nc.sync.dma_start(
                    xT_t[k][:, 512:2048],
                    xT_ext[k * 128:(k + 1) * 128, 512:2048])
            for k in range(KT):
                nc.sync.dma_start(
                    xT_t[k][:, 2048:4096],
                    xT_ext[k * 128:(k + 1) * 128, 2048:4096])
            for k in range(KT):
                nc.sync.dma_start(wv_t[k][:], wv_ext[k * 128:(k + 1) * 128, :])
            # out-proj weights: queued behind the x/qkv loads so they stream
            # in during the projection phase, long before the tail needs them.
            wout_t = [big.tile([128, D], BF16, tag=f"xT{k}", name=f"wout{k}")
                      for k in range(KT)]
            for k in range(KT):
                nc.sync.dma_start(wout_t[k][:], wout_ext[k * 128:(k + 1) * 128, :])

            # Warm the HAM clock gate during the initial xT DMA wait: ~35
            # back-to-back matmuls (~3.5us of PE activity) lift the PE to
            # 2.4 GHz before the first real matmul issues.  Output is a
            # scratch accumulator nobody reads; the source only needs a
            # cheap memset (a DMA-fed source waits on the cold DMA queue
            # even longer).
            wsrc = big.tile([128, 512], BF16, tag="wsrc")
            nc.gpsimd.memset(wsrc[:], 0.25)
            warm = pov.tile([128, 512], F32, tag="ov", name="warm")
            for i in range(35):
                nc.tensor.matmul(warm[:, 0:128], wsrc[:, 0:128],
                                 wsrc[:, 0:128],
                                 start=(i == 0), stop=(i == 34))

            # q^T/k^T images: "direct" (head0 top / head1 bottom, one fat
            # PSUM->SBUF copy) and "swapped" (halves exchanged, gpsimd).
            # Score matmul for head hl, m-tile parity h2 reads row range
            # h2*64:(h2+1)*64 of (direct if hl==h2 else swapped).
            qd_t = big.tile([128, BL], BF16, tag="qd")
            qe_t = big.tile([128, BL], BF16, tag="qe")
            kd_t = big.tile([128, BL], BF16, tag="kd")
            ke_t = big.tile([128, BL], BF16, tag="ke")
            # v: per tile [128, 2, 65]: cols h*65..h*65+63 = head h,
            # h*65+64 = ones (softmax denominator row in the PV output).
            v_t = [big.tile([128, 2, 65], BF16, tag=f"v{t}", name=f"v{t}")
                   for t in range(VT)]
            # final o^T for our 2 heads, all 4096 rows
            oT_f = big.tile([128, BL], BF16, tag="oT")

            # ---- QKV projection ----
            # All the wide q/k matmuls first (dense N=512 streams keep the
            # HAM busy through the tail of the xT DMA); the small N=128 v
            # matmuls afterwards, where they dovetail with early attention.
            def emit_qk_cols(ncols):
                # ncol pairs share each loaded wqk k-tile (the second
                # matmul's LDWEIGHTS is elided), halving weight-load cost.
                for nA, nB in zip(ncols[0::2], ncols[1::2]):
                    csA = slice(nA * 512, (nA + 1) * 512)
                    csB = slice(nB * 512, (nB + 1) * 512)
                    for m in range(2):  # 0 -> q, 1 -> k
                        psA = pov.tile([128, 512], F32, tag="ov",
                                       name=f"qk_ps{nA}_{m}")
                        psB = pov.tile([128, 512], F32, tag="ov",
                                       name=f"qk_ps{nB}_{m}")
                        for k in range(KT):
                            for ps, cs in ((psA, csA), (psB, csB)):
                                nc.tensor.matmul(
                                    ps[:],
                                    wqk_t[k][:, m * 128:(m + 1) * 128],
                                    xT_t[k][:, cs],
                                    start=(k == 0), stop=(k == KT - 1),
                                )
                        for ps, cs in ((psA, csA), (psB, csB)):
                            if m == 0:
                                nc.vector.tensor_scalar(
                                    qd_t[:, cs], ps[:],
                                    SCH_A, None, mybir.AluOpType.mult)
                            else:
                                nc.scalar.copy(kd_t[:, cs], ps[:])

            def emit_dup_half(half):
                # build the swapped q/k images for one batch's columns with
                # four fat SBUF->SBUF DMA transfers (engines stay free).
                cs = slice(half * 2048, (half + 1) * 2048)
                for dst, src_ in ((qe_t, qd_t), (ke_t, kd_t)):
                    nc.sync.dma_start(dst[0:64, cs], src_[64:128, cs])
                    nc.sync.dma_start(dst[64:128, cs], src_[0:64, cs])

            def emit_v_tiles(vts):
                for t in vts:
                    ps = pov.tile([128, 128], F32, tag="ov",
                                  name=f"v_ps{t}")
                    for k in range(KT):
                        nc.tensor.matmul(
                            ps[:],
                            xT_t[k][:, t * 128:(t + 1) * 128],
                            wv_t[k][:],
                            start=(k == 0), stop=(k == KT - 1),
                        )
                    nc.scalar.copy(
                        v_t[t][:, :, 0:64],
                        ps[:].rearrange("p (h c) -> p h c", h=2))
                    nc.vector.memset(v_t[t][:, :, 64:65], 1.0)

            # ---- attention, one (batch, head) unit at a time ----
            # hl outermost: after all hl=0 units, half of oT_f (rows 0:64)
            # is final and its AllToAll overlaps the hl=1 attention.
            cc_in = [dram.tile([N_CORES, 64, 512], BF16, name=f"cc_in{i}")
                     for i in range(2)]
            cc_out = [dram.tile([N_CORES, 64, 512], BF16, name=f"cc_out{i}")
                      for i in range(2)]
            # received head data, split by parity: ogT_e[k] rows = heads
            # (4k, 4k+2) for my 512 queries; ogT_o[k] = heads (4k+1, 4k+3).
            ogT_e = [big.tile([128, 512], BF16, tag=f"wqk{k}", name=f"ogTe{k}")
                     for k in range(KT // 2)]
            ogT_o = [big.tile([128, 512], BF16, tag=f"wqk{k + 4}", name=f"ogTo{k}")
                     for k in range(KT // 2)]

            # deferred-normalization closures (see emit_attn_unit)
            pending_norms = []

            def emit_norm(num_ap, den_sb, hs, ls):
                # den_sb: [1,512] f32 SBUF at partition 0 (custom-DVE
                # reciprocal misreads partition-shifted sources).
                rcp = ser.tile([1, 512], F32, tag="rcp")
                nc.vector.reciprocal_approx_fast(rcp[:], den_sb)
                rcpb = ser.tile([64, 512], F32, tag="rcpb")
                nc.gpsimd.partition_broadcast(rcpb[:], rcp[:], channels=64)
                nc.vector.tensor_tensor(
                    oT_f[hs, ls], num_ap, rcpb[:], mybir.AluOpType.mult)

            def emit_scores_exp(hl, b, lc):
                ls = slice(b * L + lc * 512, b * L + (lc + 1) * 512)
                pt = ptp.tile([128, MT, 512], BF16, tag="pt")
                # S^T m-tile pair per PSUM tile so exp runs at FD=1024
                # (ScalarE per-instruction overhead dominates otherwise).
                for mp in range(MT // 2):
                    st = pst.tile([128, 1024], F32, tag="st")
                    for h2 in range(2):
                        mt = 2 * mp + h2
                        rg = slice(h2 * 64, (h2 + 1) * 64)
                        qt = qd_t if hl == h2 else qe_t
                        kt_ = kd_t if hl == h2 else ke_t
                        nc.tensor.matmul(
                            st[:, h2 * 512:(h2 + 1) * 512],
                            kt_[rg, b * L + mt * 128:
                                b * L + (mt + 1) * 128],
                            qt[rg, ls],
                            start=True, stop=True,
                        )
                    if mp in DVE_MPS:
                        nc.vector.tensor_scalar(
                            pt[:, 2 * mp:2 * mp + 2, :].bitcast(
                                mybir.dt.int16),
                            st[:], SCH_B, None, mybir.AluOpType.add)
                    else:
                        nc.scalar.activation(
                            pt[:, 2 * mp:2 * mp + 2, :], st[:],
                            Exp, scale=SCH_SCALE)
                return pt

            def emit_pv_pair(ptA, ptB, hl, b, lcA, lcB, defer):
                # PV for two l-chunks of the same (head, batch), m-tile
                # major: the second matmul of each m-tile step reuses the
                # just-loaded v weights, so its LDWEIGHTS is elided.
                ovA = pov.tile([128, 512], F32, tag="ov")
                ovB = pov.tile([128, 512], F32, tag="ov")
                for mt in range(MT):
                    for ov, pt in ((ovA, ptA), (ovB, ptB)):
                        nc.tensor.matmul(
                            ov[0:65, :],
                            v_t[b * MT + mt][:, hl, :],
                            pt[:, mt, :],
                            start=(mt == 0), stop=(mt == MT - 1),
                        )
                emit_chunk_norm(ovA, hl, b, lcA, defer)
                emit_chunk_norm(ovB, hl, b, lcB, defer)

            def emit_chunk_norm(ov, hl, b, lc, defer):
                # defer=True: a collective occupies the gpsimd queue while
                # this chunk's norm would run, so its partition_broadcast
                # would stall in the gpsimd FIFO behind the collective's
                # completion wait -- and the norm multiply waiting on it
                # would stall the DVE FIFO (and everything behind it).
                # Park numerator+denominator in SBUF, free the PSUM
                # accumulator, and emit the norm ops later (drained one
                # per chunk by the next non-deferred chunks).
                hs = slice(hl * 64, (hl + 1) * 64)
                ls = slice(b * L + lc * 512, b * L + (lc + 1) * 512)
                # den row moved to an SBUF tile at partition 0 (regular
                # DVE copies handle the partition shift; the custom
                # reciprocal op does not).
                if defer:
                    den_sb = big.tile([1, 512], F32, tag=f"denp{lc}",
                                      name=f"denp{b}_{lc}")
                    nc.vector.tensor_copy(den_sb[:], ov[64:65, :])
                    num = big.tile([64, 512], F32, tag=f"ovp{lc}",
                                   name=f"ovp{b}_{lc}")
                    nc.vector.tensor_copy(num[:], ov[0:64, :])
                    pending_norms.append((num[:], den_sb[:], hs, ls))
                else:
                    if pending_norms:
                        emit_norm(*pending_norms.pop(0))
                    den_sb = small.tile([1, 512], F32, tag="den")
                    nc.vector.tensor_copy(den_sb[:], ov[64:65, :])
                    emit_norm(ov[0:64, :], den_sb[:], hs, ls)

            def emit_a2a(hl):
                # half AllToAll: rows hl*64:(hl+1)*64 of oT_f are final
                hs = slice(hl * 64, (hl + 1) * 64)
                for j in range(N_CORES):
                    nc.sync.dma_start(cc_in[hl][j],
                                      oT_f[hs, j * 512:(j + 1) * 512])
                nc.gpsimd.collective_compute(
                    "AllToAll",
                    mybir.AluOpType.bypass,
                    ins=[cc_in[hl].opt()],
                    outs=[cc_out[hl].opt()],
                    replica_groups=[list(range(N_CORES))],
                )
                # head-parity regrouping: core j's hl-half is head 2j+hl.
                # even tile k collects heads (4k, 4k+2) = cores (2k, 2k+1)
                ogT = ogT_e if hl == 0 else ogT_o
                for k in range(KT // 2):
                    nc.sync.dma_start(
                        ogT[k][:],
                        cc_out[hl][2 * k:2 * k + 2].rearrange(
                            "a b c -> (a b) c"))

            # interleave: batch-1 qkv fills PE gaps of the first
            # (ScalarE-heavy) attention unit; each half-A2A overlaps
            # the next attention units.  part tiles alias the parked-norm
            # slots (phase 1 runs only after the deferred norms drained).
            part = [big.tile([128, 512], F32,
                             tag=(f"ovp{g % 4}" if g < 4 else f"denp{g % 4}"),
                             name=f"part{g}")
                    for g in range(8)]

            def emit_phase1_group(g):
                # one even-head out-proj partial: 4 matmuls + copy to SBUF
                lt, nt = g // 2, g % 2
                ps = pst.tile([128, 512], F32, tag="st", name=f"ps1_{g}")
                for k in range(KT // 2):
                    nc.tensor.matmul(
                        ps[:],
                        ogT_e[k][:, lt * 128:(lt + 1) * 128],
                        wout_t[k][:, nt * 512:(nt + 1) * 512],
                        start=(k == 0), stop=(k == KT // 2 - 1),
                    )
                nc.scalar.copy(part[g][:], ps[:])

            emit_qk_cols(range(0, 4))
            emit_dup_half(0)
            emit_qk_cols(range(4, 8))
            emit_dup_half(1)
            emit_v_tiles(range(0, MT))
            # batch-1 v tiles double as PE filler for the exp-paced first
            # attention units.
            emit_v_tiles(range(MT, 2 * MT))
            # software-pipelined attention over l-chunk PAIRS: the PV
            # pair for chunk-pair P is emitted after the scores+exp of
            # pair P+1's first chunk, so the PV matmul chain never waits
            # on the exp engines.  The first half-A2A is emitted as soon
            # as head-0's last chunk retires.
            pairs = [(hl, b, 2 * i) for hl in (0, 1) for b in (0, 1)
                     for i in (0, 1)]
            prevp = None
            for hl, b, lc0 in pairs:
                ptA = emit_scores_exp(hl, b, lc0)
                if prevp is not None:
                    emit_pv_pair(*prevp, defer=(prevp[2] == 1 and
                                                prevp[3] == 0))
                    if prevp[2:] == (0, 1, 2, 3):
                        emit_a2a(0)
                ptB = emit_scores_exp(hl, b, lc0 + 1)
                prevp = (ptA, ptB, hl, b, lc0, lc0 + 1)
            emit_pv_pair(*prevp, defer=False)
            while pending_norms:
                emit_norm(*pending_norms.pop(0))
            # even-head out-proj partials: ready since the first AllToAll,
            # emitted here so they fill the second AllToAll's window.
            for g in range(8):
                emit_phase1_group(g)

            # Heater: keep the HAM clock gate warm across the second
            # AllToAll's transfer so phase 2 runs at full clock.
            # Reads oT_f (written by the last attention unit) so it cannot
            # be hoisted before the attention finishes; scratch accumulator.
            heat = pov.tile([128, 512], F32, tag="ov", name="heat")
            for i in range(N_HEAT):
                nc.tensor.matmul(heat[:], oT_f[:, 3584:3712],
                                 oT_f[:, 3584:4096],
                                 start=(i == 0), stop=(i == N_HEAT - 1))

            emit_a2a(1)

            # ---- out-proj phase 2: odd heads + merge with phase 1 ----
            for lt in range(4):
                for nt in range(2):
                    ps = pst.tile([128, 512], F32, tag="st")
                    for k in range(KT // 2):
                        nc.tensor.matmul(
                            ps[:],
                            ogT_o[k][:, lt * 128:(lt + 1) * 128],
                            wout_t[k + 4][:, nt * 512:(nt + 1) * 512],
                            start=(k == 0), stop=(k == KT // 2 - 1),
                        )
                    osb = small.tile([128, 512], BF16, tag="osb")
                    nc.vector.tensor_tensor(
                        osb[:], ps[:], part[lt * 2 + nt][:],
                        mybir.AluOpType.add)
                    nc.sync.dma_start(
                        out_ext[lt * 128:(lt + 1) * 128,
                                nt * 512:(nt + 1) * 512],
                        osb[:])

    nc.compile()
    return nc


_NC_CACHE = None


def _get_nc():
    global _NC_CACHE
    if _NC_CACHE is None:
        _NC_CACHE = _build()
    return _NC_CACHE


# head-parity permutation of w_out rows: heads (0,2,..,14) then (1,3,..,15)
_WOUT_PERM = np.concatenate(
    [np.arange(h * DH, (h + 1) * DH)
     for h in list(range(0, H, 2)) + list(range(1, H, 2))])


def _make_in_maps(x, w_qkv, w_out):
    x = np.asarray(x, dtype=np.float32)
    w_qkv = np.asarray(w_qkv, dtype=np.float32)
    w_out = np.asarray(w_out, dtype=np.float32)
    bf = ml_dtypes.bfloat16
    xT = np.ascontiguousarray(
        x.transpose(2, 0, 1).reshape(D, BL)).astype(bf)
    wout_b = np.ascontiguousarray(w_out[_WOUT_PERM, :]).astype(bf)
    in_maps = []
    for c in range(N_CORES):
        cs = slice(c * 128, (c + 1) * 128)
        wqk_c = np.ascontiguousarray(
            np.concatenate([w_qkv[:, cs], w_qkv[:, D:][:, cs]], axis=1)
        ).astype(bf)
        wv_c = np.ascontiguousarray(w_qkv[:, 2 * D:][:, cs]).astype(bf)
        in_maps.append({"xT": xT, "wqk": wqk_c, "wv": wv_c, "wout": wout_b})
    return in_maps


def _run(x, w_qkv, w_out, trace=False):
    nc = _get_nc()
    in_maps = _make_in_maps(x, w_qkv, w_out)
    res = bass_utils.run_bass_kernel_spmd(
        nc, in_maps, list(range(N_CORES)), trace=trace)
    out = np.empty((B, L, D), dtype=np.float32)
    for c in range(N_CORES):
        out[c // 4, (c % 4) * 512:(c % 4 + 1) * 512, :] = \
            np.asarray(res.results[c]["out"]).astype(np.float32)
    return out, res


def kernel(x, w_qkv, w_out):
    out, _ = _run(x, w_qkv, w_out, trace=False)
    return out


# revision 21
# speedup vs baseline: 1.0686x; 1.0686x over previous
"""Multi-head attention on 8 TRN2 NeuronCores.

Problem: x[2, 2048, 1024], w_qkv[1024, 3072], w_out[1024, 1024] (f32).
  qkv = x @ w_qkv; q,k,v per 16 heads of dim 64; softmax(q k^T / 8) v; out proj.

Sharding: 16 heads split 8 ways (one head-PAIR per core, both batches on
every core).  Each core computes q^T/k^T/v for its 2 heads over all
B*L = 4096 rows, runs attention, then an 8-rank AllToAll exchanges
(head-pair -> (batch, L/4-chunk)) so each core finishes the output
projection for its own 512 output rows with all 16 heads present.  The
AllToAll is split into two half-exchanges (one per local head): the first
fires halfway through attention and fully overlaps the remaining compute.

Layout: scores are computed TRANSPOSED (S^T[m, l] tiles) so softmax's sum
runs over the partition axis -- free via a ones-column appended to v in
the attn@v matmul (out rows = [o^T; colsums]).  Score matmuls come in
row-group-packed pairs: the even m-tile runs in PE rows 0:64 while the
odd m-tile runs concurrently in rows 64:128, via two q/k SBUF images --
"direct" (head0 top / head1 bottom, one fat DVE/ScalarE copy from the
projection PSUM) and "swapped" (halves exchanged by SBUF->SBUF DMA).
exp() splits across ScalarE (spline exp, 5 of 8 m-tile pairs) and the DVE
(3 of 8 via the one-op Schraudolph bit-trick); q is pre-scaled so both
read the same PSUM.  Normalization is transpose-free: the colsum row is
copied to partition 0 (the custom-DVE fast reciprocal cannot shift
partitions), reciprocal'd, broadcast over 64 partitions on gpsimd, and
multiplied into the output in one DVE op.  The attention loop is
software-pipelined: each chunk's PV matmul chain is emitted a chunk
behind its scores/exp so it never waits on the exp engines.  Chunks that
run while a collective occupies the gpsimd queue park their numerator/
denominator in SBUF and defer the norm (the broadcast would stall in the
gpsimd FIFO behind the collective and drag the DVE with it).

Tail: the out-projection is split by head parity.  Even heads arrive with
the first AllToAll, so their half of the contraction (partials parked in
SBUF, aliasing the parked-norm slots) runs during the second AllToAll's
transfer window; a chain of heater matmuls keeps the PE's HAM clock gate
warm across the collective so the odd-head half + final add run at full
clock instead of 1.2 GHz.

Compute dtype bf16 (f32 accumulation in PSUM); output returned bf16 and
upcast on host.
"""

import sys
import types

sys.path.insert(0, "/opt/trn_rl_repo")

import numpy as np
import ml_dtypes

import concourse.bass as bass
import concourse.mybir as mybir
import concourse.tile as tile
from concourse import bacc
from concourse import bass_utils

# If the image's antenv lacks the axon_hooks module, run_bass_kernel_spmd's
# trace path (reachable via BASS_TRACE=1) would die on import.  Provide the
# registry so tracing degrades gracefully instead (hook stays None unless
# trn_boot registered one).
try:
    import antenv.axon_hooks  # noqa: F401
except ImportError:
    _hooks = types.ModuleType("antenv.axon_hooks")
    _hooks._hook = None
    _hooks.set_axon_ntff_profile_hook = (
        lambda h: setattr(_hooks, "_hook", h))
    _hooks.get_axon_ntff_profile_hook = lambda: _hooks._hook
    sys.modules["antenv.axon_hooks"] = _hooks

# Artifact upload needs bucket credentials; fall back to the local dir so a
# traced run in a sandboxed container still completes.
_orig_upload = bass_utils.upload_artifacts


def _safe_upload(tmpdir):
    try:
        return _orig_upload(tmpdir)
    except Exception:
        return tmpdir


bass_utils.upload_artifacts = _safe_upload

B, L, D, H, DH = 2, 2048, 1024, 16, 64
BL = B * L  # 4096
SCALE = DH ** -0.5
N_CORES = 8
BF16 = mybir.dt.bfloat16
F32 = mybir.dt.float32
Exp = mybir.ActivationFunctionType.Exp

# exp on two engines: ScalarE evaluates the spline exp; the DVE handles the
# last DVE_MPS of every 16 key-tiles with a one-op Schraudolph bit-trick --
# bf16(int16(st + B)) ~= exp(st/A).  q is pre-scaled by A so the scores
# PSUM is already in bits-space; a uniform bits offset is a global scale
# on the softmax weights and cancels in normalization, so only the ~1.8%
# rms sawtooth remains, on the DVE-covered fraction of the keys.
SCH_A = 128 * 1.4426950408889634 * SCALE      # 23.0831
SCH_B = 16248.636                             # host-calibrated, zero mean bias
SCH_SCALE = SCALE / SCH_A                     # ScalarE: exp(st * this)
DVE_MPS = (5, 6, 7)                           # mp pairs handled by the DVE

KT = D // 128          # 8 k-tiles over the model dim
MT = L // 128          # 16 m-tiles per batch
LC = L // 512          # 4 l-chunks of 512 per batch
VT = BL // 128         # 32 v row-tiles over (b, l)
N_HEAT = 58            # heater matmuls bridging the second AllToAll


def _build():
    nc = bacc.Bacc("TRN2", target_bir_lowering=False, debug=False,
                   num_devices=N_CORES)
    xT_ext = nc.declare_dram_parameter("xT", [D, BL], BF16, isOutput=False)
    wqk_ext = nc.declare_dram_parameter("wqk", [D, 256], BF16, isOutput=False)
    wv_ext = nc.declare_dram_parameter("wv", [D, 128], BF16, isOutput=False)
    # w_out rows pre-permuted on the host: first 512 = even heads
    # (0,2,..,14), last 512 = odd heads -- so the contraction splits into
    # a half that only needs the first AllToAll and a half that needs the
    # second.
    wout_ext = nc.declare_dram_parameter("wout", [D, D], BF16, isOutput=False)
    out_ext = nc.declare_dram_parameter("out", [512, D], BF16, isOutput=True)

    with tile.TileContext(nc) as tc:
        with (
            tc.tile_pool(name="big", bufs=1) as big,
            tc.tile_pool(name="pt", bufs=3) as ptp,
            tc.tile_pool(name="small", bufs=2) as small,
            tc.tile_pool(name="ser", bufs=1) as ser,
            tc.tile_pool(name="psum_st", bufs=3, space="PSUM") as pst,
            tc.tile_pool(name="psum_ov", bufs=2, space="PSUM") as pov,
            tc.tile_pool(name="dram", bufs=1, space="DRAM") as dram,
        ):
            # ---- static SBUF tensors ----
            xT_t = [big.tile([128, BL], BF16, tag=f"xT{k}", name=f"xT{k}")
                    for k in range(KT)]
            wqk_t = [big.tile([128, 256], BF16, tag=f"wqk{k}",
                              name=f"wqk{k}") for k in range(KT)]
            wv_t = [big.tile([128, 128], BF16, tag=f"wv{k}", name=f"wv{k}")
                    for k in range(KT)]
            for k in range(KT):
                nc.sync.dma_start(
                    xT_t[k][:, 0:512], xT_ext[k * 128:(k + 1) * 128, 0:512])
                nc.sync.dma_start(wqk_t[k][:], wqk_ext[k * 128:(k + 1) * 128, :])
            # fat chunks: >=1.5KB per partition line for DMA efficiency
            for k in range(KT):
                nc.sync.dma_start(
                    xT_t[k][:, 512:2048],
                    xT_ext[k * 128:(k + 1) * 128, 512:2048])
            for k in range(KT):
                nc.sync.dma_start(
                    xT_t[k][:, 2048:4096],
                    xT_ext[k * 128:(k + 1) * 128, 2048:4096])
            for k in range(KT):
                nc.sync.dma_start(wv_t[k][:], wv_ext[k * 128:(k + 1) * 128, :])
            # out-proj weights: queued behind the x/qkv loads so they stream
            # in during the projection phase, long before the tail needs them.
            wout_t = [big.tile([128, D], BF16, tag=f"xT{k}", name=f"wout{k}")
                      for k in range(KT)]
            for k in range(KT):
                nc.sync.dma_start(wout_t[k][:], wout_ext[k * 128:(k + 1) * 128, :])

            # Warm the HAM clock gate during the initial xT DMA wait: ~35
            # back-to-back matmuls (~3.5us of PE activity) lift the PE to
            # 2.4 GHz before the first real matmul issues.  Output is a
            # scratch accumulator nobody reads; the source only needs a
            # cheap memset (a DMA-fed source waits on the cold DMA queue
            # even longer).
            wsrc = big.tile([128, 512], BF16, tag="wsrc")
            nc.gpsimd.memset(wsrc[:], 0.25)
            warm = pov.tile([128, 512], F32, tag="ov", name="warm")
            for i in range(35):
                nc.tensor.matmul(warm[:, 0:128], wsrc[:, 0:128],
                                 wsrc[:, 0:128],
                                 start=(i == 0), stop=(i == 34))

            # q^T/k^T images: "direct" (head0 top / head1 bottom, one fat
            # PSUM->SBUF copy) and "swapped" (halves exchanged, gpsimd).
            # Score matmul for head hl, m-tile parity h2 reads row range
            # h2*64:(h2+1)*64 of (direct if hl==h2 else swapped).
            qd_t = big.tile([128, BL], BF16, tag="qd")
            qe_t = big.tile([128, BL], BF16, tag="qe")
            kd_t = big.tile([128, BL], BF16, tag="kd")
            ke_t = big.tile([128, BL], BF16, tag="ke")
            # v: per tile [128, 2, 65]: cols h*65..h*65+63 = head h,
            # h*65+64 = ones (softmax denominator row in the PV output).
            v_t = [big.tile([128, 2, 65], BF16, tag=f"v{t}", name=f"v{t}")
                   for t in range(VT)]
            # final o^T for our 2 heads, all 4096 rows
            oT_f = big.tile([128, BL], BF16, tag="oT")

            # ---- QKV projection ----
            # All the wide q/k matmuls first (dense N=512 streams keep the
            # HAM busy through the tail of the xT DMA); the small N=128 v
            # matmuls afterwards, where they dovetail with early attention.
            def emit_qk_cols(ncols):
                # ncol pairs share each loaded wqk k-tile (the second
                # matmul's LDWEIGHTS is elided), halving weight-load cost.
                for nA, nB in zip(ncols[0::2], ncols[1::2]):
                    csA = slice(nA * 512, (nA + 1) * 512)
                    csB = slice(nB * 512, (nB + 1) * 512)
                    for m in range(2):  # 0 -> q, 1 -> k
                        psA = pov.tile([128, 512], F32, tag="ov",
                                       name=f"qk_ps{nA}_{m}")
                        psB = pov.tile([128, 512], F32, tag="ov",
                                       name=f"qk_ps{nB}_{m}")
                        for k in range(KT):
                            for ps, cs in ((psA, csA), (psB, csB)):
                                nc.tensor.matmul(
                                    ps[:],
                                    wqk_t[k][:, m * 128:(m + 1) * 128],
                                    xT_t[k][:, cs],
                                    start=(k == 0), stop=(k == KT - 1),
                                )
                        for ps, cs in ((psA, csA), (psB, csB)):
                            if m == 0:
                                nc.vector.tensor_scalar(
                                    qd_t[:, cs], ps[:],
                                    SCH_A, None, mybir.AluOpType.mult)
                            else:
                                nc.scalar.copy(kd_t[:, cs], ps[:])

            def emit_dup_half(half):
                # build the swapped q/k images for one batch's columns with
                # four fat SBUF->SBUF DMA transfers (engines stay free).
                cs = slice(half * 2048, (half + 1) * 2048)
                for dst, src_ in ((qe_t, qd_t), (ke_t, kd_t)):
                    nc.sync.dma_start(dst[0:64, cs], src_[64:128, cs])
                    nc.sync.dma_start(dst[64:128, cs], src_[0:64, cs])

            def emit_v_tiles(vts):
                for t in vts:
                    ps = pov.tile([128, 128], F32, tag="ov",
                                  name=f"v_ps{t}")
                    for k in range(KT):
                        nc.tensor.matmul(
                            ps[:],
                            xT_t[k][:, t * 128:(t + 1) * 128],
                            wv_t[k][:],
                            start=(k == 0), stop=(k == KT - 1),
                        )
                    nc.scalar.copy(
                        v_t[t][:, :, 0:64],
                        ps[:].rearrange("p (h c) -> p h c", h=2))
                    nc.vector.memset(v_t[t][:, :, 64:65], 1.0)

            # ---- attention, one (batch, head) unit at a time ----
            # hl outermost: after all hl=0 units, half of oT_f (rows 0:64)
            # is final and its AllToAll overlaps the hl=1 attention.
            cc_in = [dram.tile([N_CORES, 64, 512], BF16, name=f"cc_in{i}")
                     for i in range(2)]
            cc_out = [dram.tile([N_CORES, 64, 512], BF16, name=f"cc_out{i}")
                      for i in range(2)]
            # received head data, split by parity: ogT_e[k] rows = heads
            # (4k, 4k+2) for my 512 queries; ogT_o[k] = heads (4k+1, 4k+3).
            ogT_e = [big.tile([128, 512], BF16, tag=f"wqk{k}", name=f"ogTe{k}")
                     for k in range(KT // 2)]
            ogT_o = [big.tile([128, 512], BF16, tag=f"wqk{k + 4}", name=f"ogTo{k}")
                     for k in range(KT // 2)]

            # deferred-normalization closures (see emit_attn_unit)
            pending_norms = []

            def emit_norm(num_ap, den_sb, hs, ls):
                # den_sb: [1,512] f32 SBUF at partition 0 (custom-DVE
                # reciprocal misreads partition-shifted sources).
                rcp = ser.tile([1, 512], F32, tag="rcp")
                nc.vector.reciprocal_approx_fast(rcp[:], den_sb)
                rcpb = ser.tile([64, 512], F32, tag="rcpb")
                nc.gpsimd.partition_broadcast(rcpb[:], rcp[:], channels=64)
                nc.vector.tensor_tensor(
                    oT_f[hs, ls], num_ap, rcpb[:], mybir.AluOpType.mult)

            def emit_scores_exp(hl, b, lc):
                ls = slice(b * L + lc * 512, b * L + (lc + 1) * 512)
                pt = ptp.tile([128, MT, 512], BF16, tag="pt")
                # S^T m-tile pair per PSUM tile so exp runs at FD=1024
                # (ScalarE per-instruction overhead dominates otherwise).
                for mp in range(MT // 2):
                    st = pst.tile([128, 1024], F32, tag="st")
                    for h2 in range(2):
                        mt = 2 * mp + h2
                        rg = slice(h2 * 64, (h2 + 1) * 64)
                        qt = qd_t if hl == h2 else qe_t
                        kt_ = kd_t if hl == h2 else ke_t
                        nc.tensor.matmul(
                            st[:, h2 * 512:(h2 + 1) * 512],
                            kt_[rg, b * L + mt * 128:
                                b * L + (mt + 1) * 128],
                            qt[rg, ls],
                            start=True, stop=True,
                        )
                    if mp in DVE_MPS:
                        nc.vector.tensor_scalar(
                            pt[:, 2 * mp:2 * mp + 2, :].bitcast(
                                mybir.dt.int16),
                            st[:], SCH_B, None, mybir.AluOpType.add)
                    else:
                        nc.scalar.activation(
                            pt[:, 2 * mp:2 * mp + 2, :], st[:],
                            Exp, scale=SCH_SCALE)
                return pt

            def emit_pv_pair(ptA, ptB, hl, b, lcA, lcB, defer):
                # PV for two l-chunks of the same (head, batch), m-tile
                # major: the second matmul of each m-tile step reuses the
                # just-loaded v weights, so its LDWEIGHTS is elided.
                ovA = pov.tile([128, 512], F32, tag="ov")
                ovB = pov.tile([128, 512], F32, tag="ov")
                for mt in range(MT):
                    for ov, pt in ((ovA, ptA), (ovB, ptB)):
                        nc.tensor.matmul(
                            ov[0:65, :],
                            v_t[b * MT + mt][:, hl, :],
                            pt[:, mt, :],
                            start=(mt == 0), stop=(mt == MT - 1),
                        )
                emit_chunk_norm(ovA, hl, b, lcA, defer)
                emit_chunk_norm(ovB, hl, b, lcB, defer)

            def emit_chunk_norm(ov, hl, b, lc, defer):
                # defer=True: a collective occupies the gpsimd queue while
                # this chunk's norm would run, so its partition_broadcast
                # would stall in the gpsimd FIFO behind the collective's
                # completion wait -- and the norm multiply waiting on it
                # would stall the DVE FIFO (and everything behind it).
                # Park numerator+denominator in SBUF, free the PSUM
                # accumulator, and emit the norm ops later (drained one
                # per chunk by the next non-deferred chunks).
                hs = slice(hl * 64, (hl + 1) * 64)
                ls = slice(b * L + lc * 512, b * L + (lc + 1) * 512)
                # den row moved to an SBUF tile at partition 0 (regular
                # DVE copies handle the partition shift; the custom
                # reciprocal op does not).
                if defer:
                    den_sb = big.tile([1, 512], F32, tag=f"denp{lc}",
                                      name=f"denp{b}_{lc}")
                    nc.vector.tensor_copy(den_sb[:], ov[64:65, :])
                    num = big.tile([64, 512], F32, tag=f"ovp{lc}",
                                   name=f"ovp{b}_{lc}")
                    nc.vector.tensor_copy(num[:], ov[0:64, :])
                    pending_norms.append((num[:], den_sb[:], hs, ls))
                else:
                    if pending_norms:
                        emit_norm(*pending_norms.pop(0))
                    den_sb = small.tile([1, 512], F32, tag="den")
                    nc.vector.tensor_copy(den_sb[:], ov[64:65, :])
                    emit_norm(ov[0:64, :], den_sb[:], hs, ls)

            def emit_a2a(hl):
                # half AllToAll: rows hl*64:(hl+1)*64 of oT_f are final
                hs = slice(hl * 64, (hl + 1) * 64)
                for j in range(N_CORES):
                    nc.sync.dma_start(cc_in[hl][j],
                                      oT_f[hs, j * 512:(j + 1) * 512])
                nc.gpsimd.collective_compute(
                    "AllToAll",
                    mybir.AluOpType.bypass,
                    ins=[cc_in[hl].opt()],
                    outs=[cc_out[hl].opt()],
                    replica_groups=[list(range(N_CORES))],
                )
                # head-parity regrouping: core j's hl-half is head 2j+hl.
                # even tile k collects heads (4k, 4k+2) = cores (2k, 2k+1)
                ogT = ogT_e if hl == 0 else ogT_o
                for k in range(KT // 2):
                    nc.sync.dma_start(
                        ogT[k][:],
                        cc_out[hl][2 * k:2 * k + 2].rearrange(
                            "a b c -> (a b) c"))

            # interleave: batch-1 qkv fills PE gaps of the first
            # (ScalarE-heavy) attention unit; each half-A2A overlaps
            # the next attention units.  part tiles alias the parked-norm
            # slots (phase 1 runs only after the deferred norms drained).
            part = [big.tile([128, 512], F32,
                             tag=(f"ovp{g % 4}" if g < 4 else f"denp{g % 4}"),
                             name=f"part{g}")
                    for g in range(8)]

            def emit_phase1_group(g):
                # one even-head out-proj partial: 4 matmuls + copy to SBUF
                lt, nt = g // 2, g % 2
                ps = pst.tile([128, 512], F32, tag="st", name=f"ps1_{g}")
                for k in range(KT // 2):
                    nc.tensor.matmul(
                        ps[:],
                        ogT_e[k][:, lt * 128:(lt + 1) * 128],
                        wout_t[k][:, nt * 512:(nt + 1) * 512],
                        start=(k == 0), stop=(k == KT // 2 - 1),
                    )
                nc.scalar.copy(part[g][:], ps[:])

            emit_qk_cols(range(0, 4))
            emit_dup_half(0)
            emit_v_tiles(range(0, MT))
            emit_qk_cols(range(4, 8))
            emit_dup_half(1)
            # batch-1 v tiles double as PE filler for the exp-paced first
            # attention units.
            emit_v_tiles(range(MT, 2 * MT))
            # software-pipelined attention over l-chunk PAIRS: the PV
            # pair for chunk-pair P is emitted after the scores+exp of
            # pair P+1's first chunk, so the PV matmul chain never waits
            # on the exp engines.  The first half-A2A is emitted as soon
            # as head-0's last chunk retires.
            pairs = [(hl, b, 2 * i) for hl in (0, 1) for b in (0, 1)
                     for i in (0, 1)]
            prevp = None
            for hl, b, lc0 in pairs:
                ptA = emit_scores_exp(hl, b, lc0)
                if prevp is not None:
                    emit_pv_pair(*prevp, defer=(prevp[2] == 1 and
                                                prevp[3] == 0))
                    if prevp[2:] == (0, 1, 2, 3):
                        emit_a2a(0)
                ptB = emit_scores_exp(hl, b, lc0 + 1)
                prevp = (ptA, ptB, hl, b, lc0, lc0 + 1)
            emit_pv_pair(*prevp, defer=False)
            while pending_norms:
                emit_norm(*pending_norms.pop(0))
            # even-head out-proj partials: ready since the first AllToAll,
            # emitted here so they fill the second AllToAll's window.
            for g in range(8):
                emit_phase1_group(g)

            # Heater: keep the HAM clock gate warm across the second
            # AllToAll's transfer so phase 2 runs at full clock.
            # Reads oT_f (written by the last attention unit) so it cannot
            # be hoisted before the attention finishes; scratch accumulator.
            heat = pov.tile([128, 512], F32, tag="ov", name="heat")
            for i in range(N_HEAT):
                nc.tensor.matmul(heat[:], oT_f[:, 3584:3712],
                                 oT_f[:, 3584:4096],
                                 start=(i == 0), stop=(i == N_HEAT - 1))

            emit_a2a(1)

            # ---- out-proj phase 2: odd heads + merge with phase 1 ----
            for lt in range(4):
                for nt in range(2):
                    ps = pst.tile([128, 512], F32, tag="st")
                    for k in range(KT // 2):
                        nc.tensor.matmul(
                            ps[:],
                            ogT_o[k][:, lt * 128:(lt + 1) * 128],
                            wout_t[k + 4][:, nt * 512:(nt + 1) * 512],
                            start=(k == 0), stop=(k == KT // 2 - 1),
                        )
                    osb = small.tile([128, 512], BF16, tag="osb")
                    nc.vector.tensor_tensor(
                        osb[:], ps[:], part[lt * 2 + nt][:],
                        mybir.AluOpType.add)
                    nc.sync.dma_start(
                        out_ext[lt * 128:(lt + 1) * 128,
                                nt * 512:(nt + 1) * 512],
                        osb[:])

    nc.compile()
    return nc


_NC_CACHE = None


def _get_nc():
    global _NC_CACHE
    if _NC_CACHE is None:
        _NC_CACHE = _build()
    return _NC_CACHE


# head-parity permutation of w_out rows: heads (0,2,..,14) then (1,3,..,15)
_WOUT_PERM = np.concatenate(
    [np.arange(h * DH, (h + 1) * DH)
     for h in list(range(0, H, 2)) + list(range(1, H, 2))])


def _make_in_maps(x, w_qkv, w_out):
    x = np.asarray(x, dtype=np.float32)
    w_qkv = np.asarray(w_qkv, dtype=np.float32)
    w_out = np.asarray(w_out, dtype=np.float32)
    bf = ml_dtypes.bfloat16
    xT = np.ascontiguousarray(
        x.transpose(2, 0, 1).reshape(D, BL)).astype(bf)
    wout_b = np.ascontiguousarray(w_out[_WOUT_PERM, :]).astype(bf)
    in_maps = []
    for c in range(N_CORES):
        cs = slice(c * 128, (c + 1) * 128)
        wqk_c = np.ascontiguousarray(
            np.concatenate([w_qkv[:, cs], w_qkv[:, D:][:, cs]], axis=1)
        ).astype(bf)
        wv_c = np.ascontiguousarray(w_qkv[:, 2 * D:][:, cs]).astype(bf)
        in_maps.append({"xT": xT, "wqk": wqk_c, "wv": wv_c, "wout": wout_b})
    return in_maps


def _run(x, w_qkv, w_out, trace=False):
    nc = _get_nc()
    in_maps = _make_in_maps(x, w_qkv, w_out)
    res = bass_utils.run_bass_kernel_spmd(
        nc, in_maps, list(range(N_CORES)), trace=trace)
    out = np.empty((B, L, D), dtype=np.float32)
    for c in range(N_CORES):
        out[c // 4, (c % 4) * 512:(c % 4 + 1) * 512, :] = \
            np.asarray(res.results[c]["out"]).astype(np.float32)
    return out, res


def kernel(x, w_qkv, w_out):
    out, _ = _run(x, w_qkv, w_out, trace=False)
    return out
